# revision 9
# baseline (speedup 1.0000x reference)
"""Trainium2 Bass kernel for nn_BaseBranch_6811818132502 (dense_cnn).

Strategy:
 - Host-side (exact, verified vs reference): fold the channel-permutation
   einsum into conv1 weights, fold the rot90/rot-back pairs into spatially
   rotated 3x3 kernels, fold BN+conv-bias into per-channel scale/bias, and
   replace the pad-20 odd passes with pad-4 (receptive field is 5, implicit
   conv padding covers the outermost ring).  The whole module becomes:
   8 passes of [conv3x3(d=1) -> conv3x3(d=2) -> conv3x3(d=2)], each with
   fused scale/bias(+relu), then a global max over 8*64 channels, sigmoid,
   clip.
 - Device-side: data-parallel over batch (1 image per core, 8 cores).
   Channels (64) live on SBUF partitions 0:63; partitions 64:127 hold the
   same image shifted by the consuming conv's dilation (d rows), so each
   3x3 conv = 6 K=128 matmuls per output chunk (3 tap-pairs + 3 zero-padded
   singles) instead of 9 K=64 matmuls.  fp32r dtype -> 1 cycle/row on PE.
   PSUM accumulates fp32; ScalarE evicts with fused scale/bias/ReLU; DVE
   maintains the shifted upper halves and the running channel max.
"""
import sys
import os
import math

for _p in ("/opt/trn_rl_repo", "/root/.axon_site/_ro/trn_rl_repo"):
    if os.path.isdir(_p) and _p not in sys.path:
        sys.path.insert(0, _p)

import numpy as np

import concourse.bass as bass
import concourse.mybir as mybir
import concourse.tile as tile
from concourse import bacc
from concourse.bass_utils import run_bass_kernel_spmd
from contextlib import ExitStack

F32 = mybir.dt.float32
F32R = mybir.dt.float32r

BN_EPS = 1e-5
C = 64            # channels
H = W = 96        # map size
B = 8             # batch == n_cores
PAD = 4           # explicit pad for odd passes (exact; receptive field 5)
R = 4             # output rows per PSUM chunk

XO_S = H + 2 * PAD + 2      # 106: x + pad4 + conv1 halo 1
Y1_S = H + 2 * PAD          # 104: conv1 odd output domain (even uses interior)
Y2_S = H + 4                # 100: conv2 output domain (coords -2..97)

# geometry per (layer, parity): dilation, rhs row/col base offset,
# out rows/cols, output write offset into the destination buffer
GEOM = {
    (0, 0): dict(d=1, off=4, oh=96,  ow=96,  woff=4),   # conv1 even
    (0, 1): dict(d=1, off=0, oh=104, ow=104, woff=0),   # conv1 odd
    (1, 0): dict(d=2, off=2, oh=96,  ow=96,  woff=2),   # conv2 even
    (1, 1): dict(d=2, off=0, oh=100, ow=100, woff=0),   # conv2 odd
    (2, 0): dict(d=2, off=0, oh=96,  ow=96,  woff=None),  # conv3 -> ACC
    (2, 1): dict(d=2, off=0, oh=96,  ow=96,  woff=None),
}
PASS_SEQ = [0, 2, 4, 6, 1, 3, 5, 7]  # evens first: odd passes overwrite stale interiors
N_MM_SLOTS = 18  # 3 layers * 6 matmuls
W_BLOB_COLS = N_MM_SLOTS * C

_PROGRAM_CACHE = {}
TRACE = False
LAST_EXEC_NS = None


def _build_program():
    nc = bacc.Bacc("TRN2", target_bir_lowering=False, debug=False, num_devices=B)
    x_in = nc.dram_tensor("x_in", [C, H, W], F32, kind="ExternalInput")
    w_in = nc.dram_tensor("w_in", [2 * 4, 128, W_BLOB_COLS], F32, kind="ExternalInput")
    sc_in = nc.dram_tensor("sc_in", [C, 3], F32, kind="ExternalInput")
    bi_in = nc.dram_tensor("bi_in", [C, 3], F32, kind="ExternalInput")
    o_out = nc.dram_tensor("o_out", [1, H * W], F32, kind="ExternalOutput")

    with ExitStack() as ctx:
        tc = ctx.enter_context(tile.TileContext(nc))
        bigpool = ctx.enter_context(tc.tile_pool(name="big", bufs=1))
        wpool = ctx.enter_context(tc.tile_pool(name="wts", bufs=2))
        evpool = ctx.enter_context(tc.tile_pool(name="ev", bufs=6))
        psum = ctx.enter_context(tc.tile_pool(name="ps", bufs=8, space="PSUM"))

        xo = bigpool.tile([128, XO_S, XO_S], F32R)
        y1 = bigpool.tile([128, Y1_S, Y1_S], F32R)
        y2 = bigpool.tile([128, Y2_S, Y2_S], F32R)
        acc = bigpool.tile([C, H * W], F32)
        sct = bigpool.tile([C, 3], F32)
        bit = bigpool.tile([C, 3], F32)

        nc.vector.memset(xo.bitcast(F32), 0.0)
        nc.vector.memset(y1.bitcast(F32), 0.0)
        nc.vector.memset(y2.bitcast(F32), 0.0)
        nc.vector.memset(acc, 0.0)
        nc.gpsimd.dma_start(out=sct, in_=sc_in[:, :])
        nc.gpsimd.dma_start(out=bit, in_=bi_in[:, :])
        # x into the padded buffer (lower half), then build the d=1-shifted
        # upper half once (x is reused by all 8 passes)
        nc.gpsimd.dma_start(out=xo[0:C, PAD + 1:PAD + 1 + H, PAD + 1:PAD + 1 + W],
                            in_=x_in[:, :, :])
        nc.vector.tensor_copy(xo[C:128, 0:XO_S - 1, :], xo[0:C, 1:XO_S, :])

        bufs = [xo, y1, y2, None]
        strides = [XO_S, Y1_S, Y2_S]

        for pi in PASS_SEQ:
            parity = pi % 2
            wt = wpool.tile([128, W_BLOB_COLS], F32R, tag="wt")
            nc.gpsimd.dma_start(out=wt, in_=w_in[pi, :, :])

            for l in range(3):
                g = GEOM[(l, parity)]
                d, off, oh, ow, woff = g["d"], g["off"], g["oh"], g["ow"], g["woff"]
                src = bufs[l]
                dst = bufs[l + 1]
                dst_shift = 2  # upper-half row shift of the destination buffer
                h0 = 0
                while h0 < oh:
                    rr = min(R, oh - h0)
                    n = rr * ow
                    pt = psum.tile([C, 512], F32, tag="pt")
                    for j in range(6):
                        kw = j % 3
                        kh0 = 0 if j < 3 else 2  # lower-half tap row
                        rbase = h0 + kh0 * d + off
                        cbase = kw * d + off
                        lhsT = wt[:, (l * 6 + j) * C:(l * 6 + j + 1) * C]
                        rhs = src[0:128, rbase:rbase + rr, cbase:cbase + ow]
                        nc.tensor.matmul(pt[:, 0:n], lhsT, rhs,
                                         start=(j == 0), stop=(j == 5))
                    if l < 2:
                        # evict with scale/bias + relu into next buffer (lower)
                        a = h0 + woff
                        bnd = a + rr
                        nc.scalar.activation(
                            out=dst[0:C, a:bnd, woff:woff + ow],
                            in_=pt[:, 0:n].rearrange("p (r c) -> p r c", r=rr),
                            func=mybir.ActivationFunctionType.Relu,
                            bias=bit[:, l:l + 1], scale=sct[:, l:l + 1])
                        # mirror into the shifted upper half
                        u_lo = max(0, a - dst_shift)
                        u_hi = bnd - dst_shift
                        if u_hi > u_lo:
                            nc.vector.tensor_copy(
                                dst[C:128, u_lo:u_hi, :],
                                dst[0:C, u_lo + dst_shift:u_hi + dst_shift, :])
                    else:
                        # conv3: scale/bias then running channel max
                        tmp = evpool.tile([C, 512], F32, tag="ev")
                        nc.scalar.activation(
                            out=tmp[:, 0:n], in_=pt[:, 0:n],
                            func=mybir.ActivationFunctionType.Identity,
                            bias=bit[:, 2:3], scale=sct[:, 2:3])
                        nc.vector.tensor_max(
                            acc[:, h0 * W:h0 * W + n],
                            acc[:, h0 * W:h0 * W + n],
                            tmp[:, 0:n])
                    h0 += rr

        # tree max over the 64 channel partitions.  Engine APs need 32-aligned
        # partition bases and equal bases for two-SB-input ops, so stage the
        # upper half via DMA (byte-exact, any partition base) into dead y2
        # space (written through its f32r view to satisfy the f32r-producer
        # BIR check; the DMA preserves raw f32 bytes).
        ttmp_w = y2.rearrange("p a b -> p (a b)")            # f32r write view
        ttmp_r = y2.bitcast(F32).rearrange("p a b -> p (a b)")  # f32 read view
        p = 32
        while p >= 1:
            nc.gpsimd.dma_start(out=ttmp_w[0:p, 0:H * W], in_=acc[p:2 * p, :])
            nc.vector.tensor_max(acc[0:p, :], acc[0:p, :], ttmp_r[0:p, 0:H * W])
            p //= 2
        nc.scalar.activation(out=acc[0:1, :], in_=acc[0:1, :],
                             func=mybir.ActivationFunctionType.Sigmoid)
        nc.vector.tensor_scalar(acc[0:1, :], acc[0:1, :], 1e-4, 1.0 - 1e-4,
                                mybir.AluOpType.max, mybir.AluOpType.min)
        nc.gpsimd.dma_start(out=o_out[:, :], in_=acc[0:1, :])
    nc.compile()
    return nc


def _fold_weights(perms, dcn_w, dcn_b, conv2_w, conv2_b, conv3_w, conv3_b,
                  bn_gamma, bn_beta, bn_mean, bn_var):
    """Fold rotations/permutation/BN on the host. Returns (w_blob, scales, biases)."""
    scales = np.empty((C, 3), np.float32)
    biases = np.empty((C, 3), np.float32)
    conv_bs = [dcn_b, conv2_b, conv3_b]
    for l in range(3):
        s = bn_gamma[l] / np.sqrt(bn_var[l] + BN_EPS)
        scales[:, l] = s
        biases[:, l] = bn_beta[l] - bn_mean[l] * s + conv_bs[l] * s

    w_blob = np.zeros((8, 128, W_BLOB_COLS), np.float32)
    base_ws = [dcn_w, conv2_w, conv3_w]
    for i in range(8):
        k = i % 4
        for l in range(3):
            wl = np.rot90(base_ws[l], k=-k, axes=(-2, -1))
            if l == 0:
                wl = np.einsum('omhw,mj->ojhw', wl, perms[i], optimize=True)
            for j in range(6):
                kw = j % 3
                col = (l * 6 + j) * C
                if j < 3:
                    w_blob[i, 0:C, col:col + C] = wl[:, :, 0, kw].T
                    w_blob[i, C:128, col:col + C] = wl[:, :, 1, kw].T
                else:
                    w_blob[i, 0:C, col:col + C] = wl[:, :, 2, kw].T
    return w_blob, scales, biases


def kernel(x, perms, dcn_w, dcn_b, conv2_w, conv2_b, conv3_w, conv3_b,
           bn_gamma, bn_beta, bn_mean, bn_var):
    global LAST_EXEC_NS
    x = np.ascontiguousarray(np.asarray(x, np.float32))
    args = [np.asarray(a, np.float32) for a in
            (perms, dcn_w, dcn_b, conv2_w, conv2_b, conv3_w, conv3_b,
             bn_gamma, bn_beta, bn_mean, bn_var)]
    w_blob, scales, biases = _fold_weights(*args)

    if "prog" not in _PROGRAM_CACHE:
        _PROGRAM_CACHE["prog"] = _build_program()
    nc = _PROGRAM_CACHE["prog"]

    in_maps = [{
        "x_in": np.ascontiguousarray(x[b]),
        "w_in": w_blob,
        "sc_in": scales,
        "bi_in": biases,
    } for b in range(B)]

    r = run_bass_kernel_spmd(nc, in_maps, core_ids=list(range(B)), trace=TRACE)
    LAST_EXEC_NS = r.exec_time_ns
    out = np.stack([r.results[b]["o_out"].reshape(1, H, W) for b in range(B)])
    return out.astype(np.float32)
